# revision 29
# baseline (speedup 1.0000x reference)
"""Trainium2 Bass kernel for nn_Fast2Order_DE_Conv.

Math: out[b,o,ho,wo] = sum_{c,i,j} W[o, c*81+i*9+j] * p_i * p_j with
p_i = x[b, c, ho+di, wo+dj] (i = di*3+dj, 3x3 unfold of a 16-channel 64x64
image; output 62x62).

Algorithm: change the quadratic-feature basis from products p_i*p_j to
squares {p_i^2 (9), (p_i+p_j)^2 (36), i<j} per channel (720 total) and fold
the basis change into W on the host (W2).  On-chip, per spatial tile of 512
locations (8 output rows of 64 padded columns):

    5 selection matmuls (PE, f16): 576 pair-sum rows + 64 diag rows
    squares (ACT/DVE/Pool):        g = s^2, PSUM -> SBUF f16
    80 direct diag rows (DVE):     g = x*x straight from the unfold rows
    6 main matmuls (PE, f16):      out += W2T.T @ g, fp32 PSUM accumulate

The channel split is 9/7 (not 8/8): pairs are 36*9=324 and 36*7=252 rows,
so with 60/4 diag fillers the five selection chunks are exactly 128 rows
each, and the remaining 80 diag features come from x directly on the DVE
(f16 2x mode) - PE pays 11 matmul-slots per column instead of 12, and one
whole PSUM-drain is removed.  Main chunk 5 contracts only 80 rows (legal,
same cost).  Static engine assignment per tile, all under the 2347 ns PE
slot: ACT = A-pair drain + C drain + out copy (~2.25us), DVE = B-pair
PSUM->f16 copy + 2 diag muls (~1.85us), Pool = B-pair square (~2.13us).

The 3x3 unfold is free: it is the DMA access pattern (overlapping windows
of the padded l' = ho*64+wo layout).  Tiles are software-pipelined with
skew so squares get slack before the PE needs their g; warmup matmuls
during the initial DMA window ramp the PE p-state.  Outputs drain per-tile
(f32 PSUM -> packed f16) and store per-tile over the idle SP HWDGE queue,
so the tail after the last matmul is one 612ns copy + one small DMA.

Sharding: data-parallel over batch, 2 images per core on 8 cores; W-side
constants replicated; output gathered by concatenation (host upcast f32).
"""

import functools

import numpy as np

import concourse.bacc as bacc
import concourse.mybir as mybir
from concourse.tile import TileContext
from concourse.bass_utils import run_bass_kernel_spmd

B, C, H, WIDTH = 16, 16, 64, 64
O = 128
HO = WO = 62
N_CORES = 8
B_LOC = B // N_CORES
LFULL = HO * 64  # padded l' = ho*64+wo axis (3968 columns)

NCH = (9, 7)  # channel split per half
XR = (27, 21)  # unfold rows per di block (3*nch)
P36 = [(i, j) for i in range(9) for j in range(i + 1, 9)]  # 36 pairs
# Diag features: 21 from x0 rows 0..20 and 59 from x1 rows 0..58 are squared
# directly (engine partition offsets must be 32-aligned, so direct blocks
# start at x row 0 and write gD at offsets 0 and 64); the rest (x0 rows
# 21..80, x1 rows 59..62) ride the selection matmuls, filling chunks 2 and 4
# to exactly 128 rows.
NFEAT = (9 * 36 + 60, 7 * 36 + 4)  # 384, 256 sel features per half
ND = 123  # direct diag rows incl. the 43-row zero-weight gap (g chunk 5)


def _feats():
    """g row -> (c, i, j) or None (zero-weight pad), in emission order."""
    feats = []
    for c in range(9):  # half0 pairs: rows 0..323
        feats += [(c, i, j) for (i, j) in P36]
    for r in range(21, 81):  # half0 sel diag: 324..383
        i, c = divmod(r, 9)
        feats.append((c, i, i))
    for c in range(9, 16):  # half1 pairs: 384..635
        feats += [(c, i, j) for (i, j) in P36]
    for r in range(59, 63):  # half1 sel diag: 636..639
        i, c = divmod(r, 7)
        feats.append((9 + c, i, i))
    for r in range(21):  # direct diag half0: g rows 640..660
        i, c = divmod(r, 9)
        feats.append((c, i, i))
    feats += [None] * 43  # junk-but-finite gD rows 21..63: weight 0
    for r in range(59):  # direct diag half1: g rows 704..762
        i, c = divmod(r, 7)
        feats.append((9 + c, i, i))
    return feats


def _build_consts(Wf: np.ndarray):
    """W (128, 1296) -> (AselT0 [81,384], AselT1 [63,256], W2T [763,128])."""
    Wt = np.asarray(Wf, dtype=np.float64).reshape(O, C, 9, 9)
    Wsym = Wt + Wt.transpose(0, 1, 3, 2)
    feats = _feats()
    W2 = np.zeros((O, 640 + ND))
    for f, fe in enumerate(feats):
        if fe is None:
            continue
        c, i, j = fe
        if i == j:
            W2[:, f] = Wt[:, c, i, i] - 0.5 * (
                Wsym[:, c, i, :].sum(-1) - 2.0 * Wt[:, c, i, i]
            )
        else:
            W2[:, f] = 0.5 * Wsym[:, c, i, j]
    asel = []
    for h, (nch, coff) in enumerate(((9, 0), (7, 9))):
        A = np.zeros((9 * nch, NFEAT[h]), dtype=np.float32)
        base = 0 if h == 0 else NFEAT[0]
        for sf in range(NFEAT[h]):
            c, i, j = feats[base + sf]
            cl = c - coff
            A[i * nch + cl, sf] += 1.0
            if i != j:
                A[j * nch + cl, sf] += 1.0
        asel.append(A.astype(np.float16))
    W2T = np.ascontiguousarray(W2.T).astype(np.float16)  # [763, 128]
    return asel[0], asel[1], W2T


def _x_window_ap(x_d, b: int, h: int, di: int, lt_load: int):
    """Source AP for one di of the unfold load: (dj, c, l) nesting matching
    target partitions di*3*nch + dj*nch + c, free dim = padded l'."""
    nch, coff = (9, 0) if h == 0 else (7, 9)
    ap = x_d[b, coff : coff + nch, di, 0:3].unsqueeze(-1)
    v = ap.ap
    v[0] = [1, 3]
    v[1] = [H * WIDTH, nch]
    v[2] = [1, lt_load]
    return ap


def build_nc(reps: int = 1, skew: int = 2, unroll: int = 1, tw: int = 512,
             drain_mode: str = "acts", store_every: int = 2,
             n_warm: int = 9):
    """Build the per-core program.  reps>1 wraps the body in an on-chip loop
    (device-time measurement); skew is the software-pipeline depth between a
    tile's selections/squares and its main matmuls."""
    f32, f16 = mybir.dt.float32, mybir.dt.float16
    nc = bacc.Bacc("TRN2", target_bir_lowering=False)
    x_d = nc.dram_tensor("x_loc", [B_LOC, C, H, WIDTH], f16, kind="ExternalInput")
    a0_d = nc.dram_tensor("aselT0", [81, NFEAT[0]], f16, kind="ExternalInput")
    a1_d = nc.dram_tensor("aselT1", [63, NFEAT[1]], f16, kind="ExternalInput")
    w_d = nc.dram_tensor("w2T", [640 + ND, O], f16, kind="ExternalInput")
    o_d = nc.dram_tensor("out_loc", [B_LOC, O, HO * WO], f16, kind="ExternalOutput")

    rows = tw // 64
    row_tiles = []
    _ho = 0
    while _ho < HO:
        _nr = min(rows, HO - _ho)
        row_tiles.append((_ho, _nr))
        _ho += _nr

    with TileContext(nc) as tc:
        with (
            tc.tile_pool(name="const", bufs=1) as cpool,
            tc.tile_pool(name="xin", bufs=2) as xpool,
            tc.tile_pool(name="gpair", bufs=2 * (skew + 2)) as gppool,
            tc.tile_pool(name="gsng", bufs=2 * (skew + 2)) as gspool,
            tc.tile_pool(name="tmpbuf", bufs=4) as tmppool,
            tc.tile_pool(name="obuf", bufs=3) as opool,
            tc.tile_pool(name="ps_pair", bufs=2, space="PSUM") as pppool,
            tc.tile_pool(name="ps_sng", bufs=2, space="PSUM") as pspool,
            tc.tile_pool(name="ps_out", bufs=2, space="PSUM") as popool,
        ):
            a0 = cpool.tile([81, NFEAT[0]], f16, tag="a0")
            a1 = cpool.tile([63, NFEAT[1]], f16, tag="a1")
            nc.sync.dma_start(a0[:], a0_d[:])
            nc.sync.dma_start(a1[:], a1_d[:])

            def load_x(x_t, b, h, col0, col1, eng=None):
                """Fill x_t[:, col0:col1] of the unfold view for (b, half h)."""
                eng = eng or nc.sync
                nxr = XR[h]
                for di in range(3):
                    hi = min(col1, H * WIDTH - di * 64 - 2)
                    if hi > col0:
                        ap = _x_window_ap(x_d, b, h, di, hi - col0)
                        ap.offset += col0
                        eng.dma_start(x_t[di * nxr : (di + 1) * nxr, col0:hi], ap)
                    if hi < col1:
                        # pad columns feed discarded outputs; fill with
                        # arbitrary valid data to keep reads clean
                        eng.dma_start(
                            x_t[di * nxr : (di + 1) * nxr, hi:col1],
                            _x_window_ap(x_d, b, h, 0, col1 - hi),
                        )

            # Unfold loads, chunked so tiles unblock progressively and split
            # across the SP HWDGE queue and the otherwise-idle Pool SWDGE
            # queue so neither descriptor generator falls behind compute.
            xr_all = []
            for b in range(B_LOC):
                xr_b = []
                for h in range(2):
                    x_t = xpool.tile(
                        [3 * XR[h], LFULL], f16, tag=f"x{h}", name=f"x{h}_{b}"
                    )
                    xr_b.append(x_t)
                xr_all.append(xr_b)
            CH = (0, 1024, 2496, LFULL)
            for h in range(2):  # SP: batch-0 head
                load_x(xr_all[0][h], 0, h, CH[0], CH[1])
            w_r = cpool.tile([128, 5, O], f16, tag="w_r")
            nc.sync.dma_start(
                w_r[:], w_d[0:640].rearrange("(k p) o -> p k o", p=128)
            )
            w_r5 = cpool.tile([ND, O], f16, tag="w_r5")
            nc.sync.dma_start(w_r5[:], w_d[640 : 640 + ND])
            for ci in (1, 2):  # Pool SWDGE: batch-0 tail
                for h in range(2):
                    load_x(xr_all[0][h], 0, h, CH[ci], CH[ci + 1], nc.gpsimd)
            for ci in (0, 1, 2):  # SP: batch 1, in first-use order
                for b in range(1, B_LOC):
                    for h in range(2):
                        load_x(xr_all[b][h], b, h, CH[ci], CH[ci + 1])

            ogroup = {}  # batch -> (o_t, first_ho0, cols_filled)

            def do_mains(st):
                """Main matmuls + drain for a tile with g ready; store per
                group of store_every tiles over the SP HWDGE queue."""
                b, ho0, nr, g_ts = st
                gA, gC, gB, gD = g_ts
                lt = nr * 64
                ps_o = popool.tile([O, tw], f32, tag="ps_o", name="ps_o")
                moving = [
                    (w_r[:, 0, :], gA[:, 0, :lt]),
                    (w_r[:, 1, :], gA[:, 1, :lt]),
                    (w_r[:, 2, :], gC[:, :lt]),
                    (w_r[:, 3, :], gB[:, 0, :lt]),
                    (w_r[:, 4, :], gB[:, 1, :lt]),
                    (w_r5[:, :], gD[:, :lt]),
                ]
                for kk, (wv, gv) in enumerate(moving):
                    nc.tensor.matmul(
                        ps_o[:, :lt], wv, gv, start=(kk == 0), stop=(kk == 5)
                    )
                # drain fp32 PSUM -> packed f16 (dropping the 2 pad cols per
                # 64-col row) into the group store buffer
                if b not in ogroup:
                    o_t = opool.tile(
                        [O, store_every * tw], f16, tag="o", name="o_t"
                    )
                    ogroup[b] = (o_t, ho0, 0)
                o_t, g_ho0, filled = ogroup[b]
                ps_view = ps_o[:, :lt].rearrange("o (r w) -> o r w", w=64)[
                    :, :, :WO
                ]
                o_view = o_t[:, filled : filled + nr * WO].rearrange(
                    "o (r w) -> o r w", w=WO
                )
                if drain_mode == "acts":
                    nc.vector.tensor_copy(o_view, ps_view)
                else:
                    nc.scalar.copy(o_view, ps_view)
                filled += nr * WO
                if filled >= store_every * rows * WO or ho0 + nr >= HO:
                    # Pool SWDGE store: keeps the SP HWDGE queue free for
                    # the batch-1 unfold loads
                    nc.gpsimd.dma_start(
                        o_d[b, :, g_ho0 * WO : g_ho0 * WO + filled],
                        o_t[:, :filled],
                    )
                    del ogroup[b]
                else:
                    ogroup[b] = (o_t, g_ho0, filled)

            # HAM warmup: spin the PE p-state up during the initial DMA
            # window.  Dummy MMs run on a memset tile so they start without
            # waiting for any DMA; outputs are never read.
            def warmup():
                wrm = cpool.tile([128, tw], f16, tag="wrm")
                nc.vector.memset(wrm[:], 0.125)
                for i in range(n_warm):
                    ps_w = popool.tile([O, tw], f32, tag="ps_o", name="warm")
                    nc.tensor.matmul(
                        ps_w[:, :tw], wrm[:, :128], wrm[:],
                        start=True, stop=True,
                    )

            def do_sels(b, ho0, nr):
                """Selection matmuls + drains + direct diag for one tile."""
                xr = xr_all[b]
                lt = nr * 64
                c0 = ho0 * 64
                # pair A: half0 chunks 0,1 -> one 2-bank PSUM tile
                ps_A = pppool.tile([128, 2, tw], f32, tag="ps_p", name="ps_A")
                for half in range(2):
                    nc.tensor.matmul(
                        ps_A[:, half, :lt],
                        a0[:, half * 128 : (half + 1) * 128],
                        xr[0][:, c0 : c0 + lt],
                        start=True,
                        stop=True,
                    )
                gA = gppool.tile([128, 2, tw], f16, tag="gp", name="gA")
                nc.scalar.square(gA[:, :, :lt], ps_A[:, :, :lt])
                # single C: half0 chunk 2 (68 pairs + 60 diag)
                ps_C = pspool.tile([128, tw], f32, tag="ps_s", name="ps_C")
                nc.tensor.matmul(
                    ps_C[:, :lt],
                    a0[:, 256:384],
                    xr[0][:, c0 : c0 + lt],
                    start=True,
                    stop=True,
                )
                gC = gspool.tile([128, tw], f16, tag="gc", name="gC")
                tmpC = None
                if drain_mode == "bal3":
                    # copy only - the square runs on Pool from the B section
                    tmpC = tmppool.tile([128, tw], f16, tag="sq_tmpc")
                    nc.vector.tensor_copy(tmpC[:, :lt], ps_C[:, :lt])
                elif drain_mode == "bal2":
                    tmpC = tmppool.tile([128, tw], f16, tag="sq_tmpc")
                    nc.vector.tensor_copy(tmpC[:, :lt], ps_C[:, :lt])
                    nc.vector.tensor_mul(gC[:, :lt], tmpC[:, :lt], tmpC[:, :lt])
                elif drain_mode == "acts":
                    # C off the ACT queue: DVE copy (frees bank in 658ns),
                    # DVE f16 2x square off the critical path
                    tmpC = tmppool.tile([128, tw], f16, tag="sq_tmpc")
                    nc.vector.tensor_copy(tmpC[:, :lt], ps_C[:, :lt])
                    nc.vector.tensor_mul(gC[:, :lt], tmpC[:, :lt], tmpC[:, :lt])
                elif drain_mode == "split":
                    # fast bank release: DVE copies PSUM out (658ns), Pool
                    # squares the f16 tmp off the critical path
                    tmpC = tmppool.tile([128, tw], f16, tag="sq_tmpc")
                    nc.vector.tensor_copy(tmpC[:, :lt], ps_C[:, :lt])
                    nc.gpsimd.tensor_mul(gC[:, :lt], tmpC[:, :lt], tmpC[:, :lt])
                else:
                    nc.scalar.square(gC[:, :lt], ps_C[:, :lt])
                # pair B: half1 chunks 3,4 -> DVE copy + Pool square
                ps_B = pppool.tile([128, 2, tw], f32, tag="ps_p", name="ps_B")
                for half in range(2):
                    nc.tensor.matmul(
                        ps_B[:, half, :lt],
                        a1[:, half * 128 : (half + 1) * 128],
                        xr[1][:, c0 : c0 + lt],
                        start=True,
                        stop=True,
                    )
                gB = gppool.tile([128, 2, tw], f16, tag="gp", name="gB")
                if drain_mode == "bal3":
                    # bank-freeing copies lead the DVE queue; the squares go
                    # to Pool, whose latency is hidden by the mains skew
                    tmpB = tmppool.tile([128, tw], f16, tag="sq_tmpb")
                    nc.vector.tensor_copy(tmpB[:, :lt], ps_B[:, 1, :lt])
                    nc.scalar.square(gB[:, 0, :lt], ps_B[:, 0, :lt])
                    nc.gpsimd.tensor_mul(
                        gC[:, :lt], tmpC[:, :lt], tmpC[:, :lt]
                    )
                    nc.gpsimd.tensor_mul(
                        gB[:, 1, :lt], tmpB[:, :lt], tmpB[:, :lt]
                    )
                elif drain_mode == "bal2":
                    # B0 via ACT square; B1 via DVE copy + f16 2x mul
                    nc.scalar.square(gB[:, 0, :lt], ps_B[:, 0, :lt])
                    tmpB = tmppool.tile([128, tw], f16, tag="sq_tmpb")
                    nc.vector.tensor_copy(tmpB[:, :lt], ps_B[:, 1, :lt])
                    nc.vector.tensor_mul(
                        gB[:, 1, :lt], tmpB[:, :lt], tmpB[:, :lt]
                    )
                elif drain_mode == "acts":
                    # ACT direct square: bank pair frees 1038ns after the
                    # B matmuls with no cross-engine chain
                    nc.scalar.square(gB[:, :, :lt], ps_B[:, :, :lt])
                elif drain_mode == "static":
                    tmp = tmppool.tile([128, 2, tw], f16, tag="sq_tmp")
                    nc.vector.tensor_copy(tmp[:, :, :lt], ps_B[:, :, :lt])
                    nc.gpsimd.tensor_mul(
                        gB[:, :, :lt], tmp[:, :, :lt], tmp[:, :, :lt]
                    )
                elif drain_mode == "split":
                    # B0 via ACT direct square; B1 via DVE copy + DVE f16 mul
                    nc.scalar.square(gB[:, 0, :lt], ps_B[:, 0, :lt])
                    tmpB = tmppool.tile([128, tw], f16, tag="sq_tmpb")
                    nc.vector.tensor_copy(tmpB[:, :lt], ps_B[:, 1, :lt])
                    nc.gpsimd.tensor_mul(
                        gB[:, 1, :lt], tmpB[:, :lt], tmpB[:, :lt]
                    )
                else:  # "act" fallback: ACT drains everything
                    nc.scalar.square(gB[:, :, :lt], ps_B[:, :, :lt])
                # direct diag rows from x: chunk 5 (Pool in bal2 - it is
                # otherwise idle; DVE f16 2x elsewhere).  Engine partition
                # offsets must be 32-aligned: block 0 squares x0 rows 0..63
                # (only 0..20 carry weight; 21..63 are finite junk with
                # zero W2T rows), block 1 squares x1 rows 0..58 at offset 64.
                gD = gspool.tile([ND, tw], f16, tag="gd", name="gD")
                diag_eng = nc.gpsimd if drain_mode == "bal2" else nc.vector
                diag_eng.tensor_mul(
                    gD[0:64, :lt],
                    xr[0][0:64, c0 : c0 + lt],
                    xr[0][0:64, c0 : c0 + lt],
                )
                diag_eng.tensor_mul(
                    gD[64:123, :lt],
                    xr[1][0:59, c0 : c0 + lt],
                    xr[1][0:59, c0 : c0 + lt],
                )
                return gA, gC, gB, gD

            def body(it=None, unroll=1):
                # software-pipeline skew: issue tile t's selections and
                # squares, then tile (t-skew)'s mains
                pending = []
                for b in range(B_LOC):
                    for ho0, nr in row_tiles:
                        # issue ready mains BEFORE this slot's sels so a
                        # PSUM-stalled sel can't block them in the in-order
                        # PE dispatch queue
                        if len(pending) >= skew:
                            do_mains(pending.pop(0))
                        g_ts = do_sels(b, ho0, nr)
                        pending.append((b, ho0, nr, g_ts))
                for st in pending:
                    do_mains(st)

            warmup()
            if reps == 1:
                for _ in range(unroll):
                    body()
            else:
                hint = (
                    mybir.EngineType.PE,
                    mybir.EngineType.Activation,
                    mybir.EngineType.DVE,
                    mybir.EngineType.SP,
                    mybir.EngineType.Pool,
                )
                with tc.For_i(0, reps, 1, hint_engines=hint) as _it:
                    for _ in range(unroll):
                        body()
    nc.compile()
    return nc


@functools.lru_cache(maxsize=1)
def _cached_nc():
    return build_nc()


def _core_in_map(x_r, consts, k):
    """Per-core input map (shared with probe_hwtime)."""
    AselT0, AselT1, W2T = consts
    return {
        "x_loc": np.ascontiguousarray(x_r[k * B_LOC : (k + 1) * B_LOC]),
        "aselT0": AselT0,
        "aselT1": AselT1,
        "w2T": W2T,
    }


def kernel(x: np.ndarray, W: np.ndarray, _trace: bool = False):
    x = np.asarray(x, dtype=np.float32)
    W = np.asarray(W, dtype=np.float32)
    consts = _build_consts(W)
    x_r = x.astype(np.float16)

    nc = _cached_nc()
    in_maps = [_core_in_map(x_r, consts, k) for k in range(N_CORES)]
    try:
        r = run_bass_kernel_spmd(
            nc, in_maps, core_ids=list(range(N_CORES)), trace=_trace
        )
    except Exception:
        # transient NRT_EXEC_UNIT_UNRECOVERABLE has been observed on this
        # fabric; a fresh attempt recovers
        r = run_bass_kernel_spmd(
            nc, in_maps, core_ids=list(range(N_CORES)), trace=_trace
        )
    out = np.concatenate([m["out_loc"] for m in r.results], axis=0)
    out = out.astype(np.float32).reshape(B, O, HO, WO)
    if _trace:
        kernel.last_result = r
    return out


if __name__ == "__main__":
    rng = np.random.default_rng(0)
    x = rng.standard_normal((B, C, H, WIDTH), dtype=np.float32)
    W = rng.standard_normal((O, C * 81), dtype=np.float32)
    out = kernel(x, W)
    print("out shape", out.shape, out.dtype)


# revision 38
# speedup vs baseline: 1.0416x; 1.0416x over previous
"""Trainium2 Bass kernel for nn_Fast2Order_DE_Conv.

Math: out[b,o,ho,wo] = sum_{c,i,j} W[o, c*81+i*9+j] * p_i * p_j with
p_i = x[b, c, ho+di, wo+dj] (i = di*3+dj, 3x3 unfold of a 16-channel 64x64
image; output 62x62).

Algorithm: change the quadratic-feature basis from products p_i*p_j to
squares {p_i^2 (9), (p_i+p_j)^2 (36), i<j} per channel (720 total) and fold
the basis change into W on the host (W2).  On-chip, per spatial tile of 512
locations (8 output rows of 64 padded columns):

    5 selection matmuls (PE, f16): 576 pair-sum rows + 64 diag rows
    squares (ACT/DVE/Pool):        g = s^2, PSUM -> SBUF f16
    80 direct diag rows (DVE):     g = x*x straight from the unfold rows
    6 main matmuls (PE, f16):      out += W2T.T @ g, fp32 PSUM accumulate

The channel split is 9/7 (not 8/8): pairs are 36*9=324 and 36*7=252 rows,
so with 60/4 diag fillers the five selection chunks are exactly 128 rows
each, and the remaining 80 diag features come from x directly on the DVE
(f16 2x mode) - PE pays 11 matmul-slots per column instead of 12, and one
whole PSUM-drain is removed.  Main chunk 5 contracts only 80 rows (legal,
same cost).  Static engine assignment per tile, all under the 2347 ns PE
slot: ACT = A-pair drain + C drain + out copy (~2.25us), DVE = B-pair
PSUM->f16 copy + 2 diag muls (~1.85us), Pool = B-pair square (~2.13us).

The 3x3 unfold is free: it is the DMA access pattern (overlapping windows
of the padded l' = ho*64+wo layout).  Tiles are software-pipelined with
skew so squares get slack before the PE needs their g; warmup matmuls
during the initial DMA window ramp the PE p-state.  Outputs drain per-tile
(f32 PSUM -> packed f16) and store per-tile over the idle SP HWDGE queue,
so the tail after the last matmul is one 612ns copy + one small DMA.

Sharding: data-parallel over batch, 2 images per core on 8 cores; W-side
constants replicated; output gathered by concatenation (host upcast f32).
"""

import functools

import numpy as np

import concourse.bacc as bacc
import concourse.mybir as mybir
from concourse.tile import TileContext
from concourse.bass_utils import run_bass_kernel_spmd

B, C, H, WIDTH = 16, 16, 64, 64
O = 128
HO = WO = 62
N_CORES = 8
B_LOC = B // N_CORES
LFULL = HO * 64  # padded l' = ho*64+wo axis (3968 columns)

NCH = (9, 7)  # channel split per half
XR = (27, 21)  # unfold rows per di block (3*nch)
P36 = [(i, j) for i in range(9) for j in range(i + 1, 9)]  # 36 pairs
# Diag features: 21 from x0 rows 0..20 and 59 from x1 rows 0..58 are squared
# directly (engine partition offsets must be 32-aligned, so direct blocks
# start at x row 0 and write gD at offsets 0 and 64); the rest (x0 rows
# 21..80, x1 rows 59..62) ride the selection matmuls, filling chunks 2 and 4
# to exactly 128 rows.
NFEAT = (9 * 36 + 60, 7 * 36 + 4)  # 384, 256 sel features per half
ND = 123  # direct diag rows incl. the 43-row zero-weight gap (g chunk 5)


def _feats():
    """g row -> (c, i, j) or None (zero-weight pad), in emission order."""
    feats = []
    for c in range(9):  # half0 pairs: rows 0..323
        feats += [(c, i, j) for (i, j) in P36]
    for r in range(21, 81):  # half0 sel diag: 324..383
        i, c = divmod(r, 9)
        feats.append((c, i, i))
    for c in range(9, 16):  # half1 pairs: 384..635
        feats += [(c, i, j) for (i, j) in P36]
    for r in range(59, 63):  # half1 sel diag: 636..639
        i, c = divmod(r, 7)
        feats.append((9 + c, i, i))
    for r in range(21):  # direct diag half0: g rows 640..660
        i, c = divmod(r, 9)
        feats.append((c, i, i))
    feats += [None] * 43  # junk-but-finite gD rows 21..63: weight 0
    for r in range(59):  # direct diag half1: g rows 704..762
        i, c = divmod(r, 7)
        feats.append((9 + c, i, i))
    return feats


def _build_consts(Wf: np.ndarray):
    """W (128, 1296) -> (AselT0 [81,384], AselT1 [63,256], W2T [763,128])."""
    Wt = np.asarray(Wf, dtype=np.float64).reshape(O, C, 9, 9)
    Wsym = Wt + Wt.transpose(0, 1, 3, 2)
    feats = _feats()
    W2 = np.zeros((O, 640 + ND))
    for f, fe in enumerate(feats):
        if fe is None:
            continue
        c, i, j = fe
        if i == j:
            W2[:, f] = Wt[:, c, i, i] - 0.5 * (
                Wsym[:, c, i, :].sum(-1) - 2.0 * Wt[:, c, i, i]
            )
        else:
            W2[:, f] = 0.5 * Wsym[:, c, i, j]
    asel = []
    for h, (nch, coff) in enumerate(((9, 0), (7, 9))):
        A = np.zeros((9 * nch, NFEAT[h]), dtype=np.float32)
        base = 0 if h == 0 else NFEAT[0]
        for sf in range(NFEAT[h]):
            c, i, j = feats[base + sf]
            cl = c - coff
            A[i * nch + cl, sf] += 1.0
            if i != j:
                A[j * nch + cl, sf] += 1.0
        asel.append(A.astype(np.float16))
    W2T = np.ascontiguousarray(W2.T).astype(np.float16)  # [763, 128]
    return asel[0], asel[1], W2T


def _x_window_ap(x_d, b: int, h: int, di: int, lt_load: int):
    """Source AP for one di of the unfold load: (dj, c, l) nesting matching
    target partitions di*3*nch + dj*nch + c, free dim = padded l'."""
    nch, coff = (9, 0) if h == 0 else (7, 9)
    ap = x_d[b, coff : coff + nch, di, 0:3].unsqueeze(-1)
    v = ap.ap
    v[0] = [1, 3]
    v[1] = [H * WIDTH, nch]
    v[2] = [1, lt_load]
    return ap


def build_nc(reps: int = 1, skew: int = 2, unroll: int = 1, tw: int = 512,
             drain_mode: str = "acts", store_every: int = 2,
             n_warm: int = 9):
    """Build the per-core program.  reps>1 wraps the body in an on-chip loop
    (device-time measurement); skew is the software-pipeline depth between a
    tile's selections/squares and its main matmuls."""
    f32, f16 = mybir.dt.float32, mybir.dt.float16
    nc = bacc.Bacc("TRN2", target_bir_lowering=False)
    x_d = nc.dram_tensor("x_loc", [B_LOC, C, H, WIDTH], f16, kind="ExternalInput")
    a0_d = nc.dram_tensor("aselT0", [81, NFEAT[0]], f16, kind="ExternalInput")
    a1_d = nc.dram_tensor("aselT1", [63, NFEAT[1]], f16, kind="ExternalInput")
    w_d = nc.dram_tensor("w2T", [640 + ND, O], f16, kind="ExternalInput")
    o_d = nc.dram_tensor("out_loc", [B_LOC, O, HO * WO], f16, kind="ExternalOutput")

    rows = tw // 64
    row_tiles = []
    _ho = 0
    while _ho < HO:
        _nr = min(rows, HO - _ho)
        row_tiles.append((_ho, _nr))
        _ho += _nr

    with TileContext(nc) as tc:
        with (
            tc.tile_pool(name="const", bufs=1) as cpool,
            tc.tile_pool(name="xin", bufs=2) as xpool,
            tc.tile_pool(name="gpair", bufs=2 * (skew + 2)) as gppool,
            tc.tile_pool(name="gsng", bufs=2 * (skew + 2)) as gspool,
            tc.tile_pool(name="tmpbuf", bufs=4) as tmppool,
            tc.tile_pool(name="obuf", bufs=3) as opool,
            tc.tile_pool(name="ps_pair", bufs=2, space="PSUM") as pppool,
            tc.tile_pool(name="ps_sng", bufs=2, space="PSUM") as pspool,
            tc.tile_pool(name="ps_out", bufs=2, space="PSUM") as popool,
        ):
            a0 = cpool.tile([81, NFEAT[0]], f16, tag="a0")
            a1 = cpool.tile([63, NFEAT[1]], f16, tag="a1")
            nc.sync.dma_start(a0[:], a0_d[:])
            nc.sync.dma_start(a1[:], a1_d[:])

            def load_x(x_t, b, h, col0, col1, eng=None):
                """Fill x_t[:, col0:col1] of the unfold view for (b, half h)."""
                eng = eng or nc.sync
                nxr = XR[h]
                for di in range(3):
                    hi = min(col1, H * WIDTH - di * 64 - 2)
                    if hi > col0:
                        ap = _x_window_ap(x_d, b, h, di, hi - col0)
                        ap.offset += col0
                        eng.dma_start(x_t[di * nxr : (di + 1) * nxr, col0:hi], ap)
                    if hi < col1:
                        # pad columns feed discarded outputs; fill with
                        # arbitrary valid data to keep reads clean
                        eng.dma_start(
                            x_t[di * nxr : (di + 1) * nxr, hi:col1],
                            _x_window_ap(x_d, b, h, 0, col1 - hi),
                        )

            # Unfold loads, chunked so tiles unblock progressively and split
            # across the SP HWDGE queue and the otherwise-idle Pool SWDGE
            # queue so neither descriptor generator falls behind compute.
            xr_all = []
            for b in range(B_LOC):
                xr_b = []
                for h in range(2):
                    x_t = xpool.tile(
                        [3 * XR[h], LFULL], f16, tag=f"x{h}", name=f"x{h}_{b}"
                    )
                    xr_b.append(x_t)
                xr_all.append(xr_b)
            CH = (0, 1536, 2752, LFULL)
            for h in range(2):  # SP: batch-0 head
                load_x(xr_all[0][h], 0, h, CH[0], CH[1])
            w_r = cpool.tile([128, 5, O], f16, tag="w_r")
            nc.sync.dma_start(
                w_r[:], w_d[0:640].rearrange("(k p) o -> p k o", p=128)
            )
            w_r5 = cpool.tile([ND, O], f16, tag="w_r5")
            nc.sync.dma_start(w_r5[:], w_d[640 : 640 + ND])
            # batch-0 tail: SP whenever Pool has per-tile compute (its SWDGE
            # generation would block the early diag squares)
            tail_eng = (
                nc.sync if drain_mode in ("bal4", "acts") else nc.gpsimd
            )
            for ci in (1, 2):
                for h in range(2):
                    load_x(xr_all[0][h], 0, h, CH[ci], CH[ci + 1], tail_eng)
            # batch 1: emitted lazily at tile boundaries (single-shot) so
            # stores interleave on the SP queue; pre-issued in reps mode
            def _b1_chunk(ci):
                def emit():
                    for b in range(1, B_LOC):
                        for h in range(2):
                            load_x(xr_all[b][h], b, h, CH[ci], CH[ci + 1])
                return emit
            deferred = {2: _b1_chunk(0), 4: _b1_chunk(1), 6: _b1_chunk(2)}
            if reps != 1:
                for ci in sorted(deferred):
                    deferred[ci]()
                deferred = {}

            ogroup = {}  # batch -> (o_t, first_ho0, cols_filled)

            def do_mains(st):
                """Main matmuls + drain for a tile with g ready; store per
                group of store_every tiles over the SP HWDGE queue."""
                b, ho0, nr, g_ts = st
                gA, gC, gB, gD = g_ts
                lt = nr * 64
                ps_o = popool.tile([O, tw], f32, tag="ps_o", name="ps_o")
                moving = [
                    (w_r[:, 0, :], gA[:, 0, :lt]),
                    (w_r[:, 1, :], gA[:, 1, :lt]),
                    (w_r[:, 2, :], gC[:, :lt]),
                    (w_r[:, 3, :], gB[:, 0, :lt]),
                    (w_r[:, 4, :], gB[:, 1, :lt]),
                    (w_r5[:, :], gD[:, :lt]),
                ]
                for kk, (wv, gv) in enumerate(moving):
                    nc.tensor.matmul(
                        ps_o[:, :lt], wv, gv, start=(kk == 0), stop=(kk == 5)
                    )
                # drain fp32 PSUM -> packed f16 (dropping the 2 pad cols per
                # 64-col row) into the group store buffer
                if b not in ogroup:
                    o_t = opool.tile(
                        [O, store_every * tw], f16, tag="o", name="o_t"
                    )
                    ogroup[b] = (o_t, ho0, 0)
                o_t, g_ho0, filled = ogroup[b]
                ps_view = ps_o[:, :lt].rearrange("o (r w) -> o r w", w=64)[
                    :, :, :WO
                ]
                o_view = o_t[:, filled : filled + nr * WO].rearrange(
                    "o (r w) -> o r w", w=WO
                )
                if drain_mode == "acts":
                    nc.vector.tensor_copy(o_view, ps_view)
                else:
                    nc.scalar.copy(o_view, ps_view)
                filled += nr * WO
                if filled >= store_every * rows * WO or ho0 + nr >= HO:
                    # when Pool has per-tile compute the stores ride the SP
                    # HWDGE queue; otherwise Pool SWDGE keeps SP free for
                    # the batch-1 unfold loads
                    store_eng = (
                        nc.sync
                        if drain_mode in ("bal4", "acts")
                        else nc.gpsimd
                    )
                    store_eng.dma_start(
                        o_d[b, :, g_ho0 * WO : g_ho0 * WO + filled],
                        o_t[:, :filled],
                    )
                    del ogroup[b]
                else:
                    ogroup[b] = (o_t, g_ho0, filled)

            # HAM warmup: spin the PE p-state up during the initial DMA
            # window.  Dummy MMs run on a memset tile so they start without
            # waiting for any DMA; outputs are never read.
            def warmup():
                wrm = cpool.tile([128, tw], f16, tag="wrm")
                nc.vector.memset(wrm[:], 0.125)
                for i in range(n_warm):
                    ps_w = popool.tile([O, tw], f32, tag="ps_o", name="warm")
                    nc.tensor.matmul(
                        ps_w[:, :tw], wrm[:, :128], wrm[:],
                        start=True, stop=True,
                    )

            def do_sels(b, ho0, nr):
                """Selection matmuls + drains + direct diag for one tile."""
                xr = xr_all[b]
                lt = nr * 64
                c0 = ho0 * 64
                # pair A: half0 chunks 0,1 -> one 2-bank PSUM tile
                ps_A = pppool.tile([128, 2, tw], f32, tag="ps_p", name="ps_A")
                for half in range(2):
                    nc.tensor.matmul(
                        ps_A[:, half, :lt],
                        a0[:, half * 128 : (half + 1) * 128],
                        xr[0][:, c0 : c0 + lt],
                        start=True,
                        stop=True,
                    )
                gA = gppool.tile([128, 2, tw], f16, tag="gp", name="gA")
                nc.scalar.square(gA[:, :, :lt], ps_A[:, :, :lt])
                # single C: half0 chunk 2 (68 pairs + 60 diag)
                ps_C = pspool.tile([128, tw], f32, tag="ps_s", name="ps_C")
                nc.tensor.matmul(
                    ps_C[:, :lt],
                    a0[:, 256:384],
                    xr[0][:, c0 : c0 + lt],
                    start=True,
                    stop=True,
                )
                gC = gspool.tile([128, tw], f16, tag="gc", name="gC")
                tmpC = None
                if drain_mode == "bal4":
                    nc.scalar.square(gC[:, :lt], ps_C[:, :lt])
                elif drain_mode == "bal3":
                    # copy only - the square runs on Pool from the B section
                    tmpC = tmppool.tile([128, tw], f16, tag="sq_tmpc")
                    nc.vector.tensor_copy(tmpC[:, :lt], ps_C[:, :lt])
                elif drain_mode == "bal2":
                    tmpC = tmppool.tile([128, tw], f16, tag="sq_tmpc")
                    nc.vector.tensor_copy(tmpC[:, :lt], ps_C[:, :lt])
                    nc.vector.tensor_mul(gC[:, :lt], tmpC[:, :lt], tmpC[:, :lt])
                elif drain_mode == "acts":
                    # C off the ACT queue: DVE copy (frees bank in 658ns),
                    # DVE f16 2x square off the critical path
                    tmpC = tmppool.tile([128, tw], f16, tag="sq_tmpc")
                    nc.vector.tensor_copy(tmpC[:, :lt], ps_C[:, :lt])
                    nc.vector.tensor_mul(gC[:, :lt], tmpC[:, :lt], tmpC[:, :lt])
                elif drain_mode == "split":
                    # fast bank release: DVE copies PSUM out (658ns), Pool
                    # squares the f16 tmp off the critical path
                    tmpC = tmppool.tile([128, tw], f16, tag="sq_tmpc")
                    nc.vector.tensor_copy(tmpC[:, :lt], ps_C[:, :lt])
                    nc.gpsimd.tensor_mul(gC[:, :lt], tmpC[:, :lt], tmpC[:, :lt])
                else:
                    nc.scalar.square(gC[:, :lt], ps_C[:, :lt])
                # pair B: half1 chunks 3,4 -> DVE copy + Pool square
                ps_B = pppool.tile([128, 2, tw], f32, tag="ps_p", name="ps_B")
                for half in range(2):
                    nc.tensor.matmul(
                        ps_B[:, half, :lt],
                        a1[:, half * 128 : (half + 1) * 128],
                        xr[1][:, c0 : c0 + lt],
                        start=True,
                        stop=True,
                    )
                gB = gppool.tile([128, 2, tw], f16, tag="gp", name="gB")
                if drain_mode == "bal4":
                    # whole B pair via DVE copy + mul (sized for 1x DVE f16)
                    tmp = tmppool.tile([128, 2, tw], f16, tag="sq_tmp")
                    nc.vector.tensor_copy(tmp[:, :, :lt], ps_B[:, :, :lt])
                    nc.vector.tensor_mul(
                        gB[:, :, :lt], tmp[:, :, :lt], tmp[:, :, :lt]
                    )
                elif drain_mode == "bal3":
                    # bank-freeing copies lead the DVE queue; the squares go
                    # to Pool, whose latency is hidden by the mains skew
                    tmpB = tmppool.tile([128, tw], f16, tag="sq_tmpb")
                    nc.vector.tensor_copy(tmpB[:, :lt], ps_B[:, 1, :lt])
                    nc.scalar.square(gB[:, 0, :lt], ps_B[:, 0, :lt])
                    nc.gpsimd.tensor_mul(
                        gC[:, :lt], tmpC[:, :lt], tmpC[:, :lt]
                    )
                    nc.gpsimd.tensor_mul(
                        gB[:, 1, :lt], tmpB[:, :lt], tmpB[:, :lt]
                    )
                elif drain_mode == "bal2":
                    # B0 via ACT square; B1 via DVE copy + f16 2x mul
                    nc.scalar.square(gB[:, 0, :lt], ps_B[:, 0, :lt])
                    tmpB = tmppool.tile([128, tw], f16, tag="sq_tmpb")
                    nc.vector.tensor_copy(tmpB[:, :lt], ps_B[:, 1, :lt])
                    nc.vector.tensor_mul(
                        gB[:, 1, :lt], tmpB[:, :lt], tmpB[:, :lt]
                    )
                elif drain_mode == "acts":
                    # ACT direct square: bank pair frees 1038ns after the
                    # B matmuls with no cross-engine chain
                    nc.scalar.square(gB[:, :, :lt], ps_B[:, :, :lt])
                elif drain_mode == "static":
                    tmp = tmppool.tile([128, 2, tw], f16, tag="sq_tmp")
                    nc.vector.tensor_copy(tmp[:, :, :lt], ps_B[:, :, :lt])
                    nc.gpsimd.tensor_mul(
                        gB[:, :, :lt], tmp[:, :, :lt], tmp[:, :, :lt]
                    )
                elif drain_mode == "split":
                    # B0 via ACT direct square; B1 via DVE copy + DVE f16 mul
                    nc.scalar.square(gB[:, 0, :lt], ps_B[:, 0, :lt])
                    tmpB = tmppool.tile([128, tw], f16, tag="sq_tmpb")
                    nc.vector.tensor_copy(tmpB[:, :lt], ps_B[:, 1, :lt])
                    nc.gpsimd.tensor_mul(
                        gB[:, 1, :lt], tmpB[:, :lt], tmpB[:, :lt]
                    )
                else:  # "act" fallback: ACT drains everything
                    nc.scalar.square(gB[:, :, :lt], ps_B[:, :, :lt])
                # direct diag rows from x: chunk 5 (Pool in bal2 - it is
                # otherwise idle; DVE f16 2x elsewhere).  Engine partition
                # offsets must be 32-aligned: block 0 squares x0 rows 0..63
                # (only 0..20 carry weight; 21..63 are finite junk with
                # zero W2T rows), block 1 squares x1 rows 0..58 at offset 64.
                gD = gspool.tile([ND, tw], f16, tag="gd", name="gD")
                d_eng = {
                    "bal2": (nc.gpsimd, nc.gpsimd),
                    "bal4": (nc.gpsimd, nc.gpsimd),
                    # acts: split - DVE has room for one, Pool takes the other
                    "acts": (nc.vector, nc.gpsimd),
                }.get(drain_mode, (nc.vector, nc.vector))
                d_eng[0].tensor_mul(
                    gD[0:64, :lt],
                    xr[0][0:64, c0 : c0 + lt],
                    xr[0][0:64, c0 : c0 + lt],
                )
                d_eng[1].tensor_mul(
                    gD[64:123, :lt],
                    xr[1][0:59, c0 : c0 + lt],
                    xr[1][0:59, c0 : c0 + lt],
                )
                return gA, gC, gB, gD

            def body(it=None, unroll=1):
                # software-pipeline skew: issue tile t's selections and
                # squares, then tile (t-skew)'s mains
                pending = []
                t_idx = 0
                for b in range(B_LOC):
                    for ho0, nr in row_tiles:
                        if t_idx in deferred:
                            deferred.pop(t_idx)()
                        # issue ready mains BEFORE this slot's sels so a
                        # PSUM-stalled sel can't block them in the in-order
                        # PE dispatch queue
                        if len(pending) >= skew:
                            do_mains(pending.pop(0))
                        g_ts = do_sels(b, ho0, nr)
                        pending.append((b, ho0, nr, g_ts))
                        t_idx += 1
                for st in pending:
                    do_mains(st)

            warmup()
            if reps == 1:
                for _ in range(unroll):
                    body()
            else:
                hint = (
                    mybir.EngineType.PE,
                    mybir.EngineType.Activation,
                    mybir.EngineType.DVE,
                    mybir.EngineType.SP,
                    mybir.EngineType.Pool,
                )
                with tc.For_i(0, reps, 1, hint_engines=hint) as _it:
                    for _ in range(unroll):
                        body()
    nc.compile()
    return nc


@functools.lru_cache(maxsize=1)
def _cached_nc():
    return build_nc()


def _core_in_map(x_r, consts, k):
    """Per-core input map (shared with probe_hwtime)."""
    AselT0, AselT1, W2T = consts
    return {
        "x_loc": np.ascontiguousarray(x_r[k * B_LOC : (k + 1) * B_LOC]),
        "aselT0": AselT0,
        "aselT1": AselT1,
        "w2T": W2T,
    }


def kernel(x: np.ndarray, W: np.ndarray, _trace: bool = False):
    x = np.asarray(x, dtype=np.float32)
    W = np.asarray(W, dtype=np.float32)
    consts = _build_consts(W)
    x_r = x.astype(np.float16)

    nc = _cached_nc()
    in_maps = [_core_in_map(x_r, consts, k) for k in range(N_CORES)]
    try:
        r = run_bass_kernel_spmd(
            nc, in_maps, core_ids=list(range(N_CORES)), trace=_trace
        )
    except Exception:
        # transient NRT_EXEC_UNIT_UNRECOVERABLE has been observed on this
        # fabric; a fresh attempt recovers
        r = run_bass_kernel_spmd(
            nc, in_maps, core_ids=list(range(N_CORES)), trace=_trace
        )
    out = np.concatenate([m["out_loc"] for m in r.results], axis=0)
    out = out.astype(np.float32).reshape(B, O, HO, WO)
    if _trace:
        kernel.last_result = r
    return out


if __name__ == "__main__":
    rng = np.random.default_rng(0)
    x = rng.standard_normal((B, C, H, WIDTH), dtype=np.float32)
    W = rng.standard_normal((O, C * 81), dtype=np.float32)
    out = kernel(x, W)
    print("out shape", out.shape, out.dtype)
